# revision 11
# baseline (speedup 1.0000x reference)
"""Segment-reduce BatchNorm (scalar + vector branches) on 8 TRN2 NeuronCores.

Strategy (per sharding hint): split the 512 sorted segments into 8 blocks of
64 contiguous segments; each core gets the nodes of its 64 segments, so all
segment statistics are device-local (no collectives).

Single-pass design: s and v stream through SBUF exactly once (fp32).  Node
tiles are processed in chunks; per-segment sums accumulate in PSUM via
one-hot bf16 matmuls while each chunk stays resident in SBUF.  Because batch
is sorted and a segment is far smaller than a chunk, the statistics of every
segment appearing in chunk c are final once chunk c+1's sums are in, so the
pipeline is: stats(c+1) -> finalize(c) -> apply(c), with apply reading the
retained SBUF tiles (no second DRAM read of s/v).

ACT casts s/v to bf16 and squares them; PE does all segment reductions and
per-node expansion of the per-segment scale/offset tables (bf16 one-hot
matmuls); DVE applies the normalization in fp32.

All host work is index/layout-only (sharding, padding, one-hot build, layout
transforms); every reduction and normalization happens on device.
"""

import numpy as np
import ml_dtypes

bf16 = ml_dtypes.bfloat16

B = 512           # total segments
NCORES = 8
BL = B // NCORES  # 64 segments per core
NT = 26624        # padded nodes per core (208 tiles of 128)
T = NT // 128     # 208 node tiles
K = 4             # node tiles per DMA supertile
ST = T // K       # 52 supertiles
CH = 8            # supertiles per chunk (4096 nodes >> max segment size)
NCHUNK = (ST + CH - 1) // CH
SDIM = 256
VD = 192          # 64 channels x 3 components, staged component-major
EPS = 1e-6
SV_W = SDIM + VD  # 448 cols per node: s | v
SCW = 576         # scratch cols per node: s16 | s2 | vsq

_compiled = None


def _emit(ctx, tc, nc, mybir, d_sv, d_p1, d_p3, d_ci, d_wb, d_out):
    import concourse.bass as bass

    f32 = mybir.dt.float32
    b16 = mybir.dt.bfloat16
    ts = bass.ts
    Act = mybir.ActivationFunctionType

    const_pool = ctx.enter_context(tc.tile_pool(name="const", bufs=1))
    cnti = const_pool.tile([BL, 1], f32)
    nc.sync.dma_start(cnti[:], d_ci[:])
    wbt = const_pool.tile([BL, 2 * SDIM], f32)
    nc.sync.dma_start(wbt[:], d_wb[:])
    ci = cnti[:, 0:1]

    # PSUM banks: stats 2 + psac 2x2 + psr 2x1 = 8
    pstat = ctx.enter_context(tc.tile_pool(name="pstat", bufs=1, space="PSUM"))
    ps_ss2 = pstat.tile([BL, 512], f32, tag="ps_ss2")   # sum s | sum s^2
    ps_v = pstat.tile([BL, BL], f32, tag="ps_v")        # sum |v|^2 per channel

    sv_pool = ctx.enter_context(tc.tile_pool(name="sv", bufs=2 * CH + 4))
    p1_pool = ctx.enter_context(tc.tile_pool(name="p1", bufs=3))
    p3_pool = ctx.enter_context(tc.tile_pool(name="p3", bufs=3))
    scr_pool = ctx.enter_context(tc.tile_pool(name="scr", bufs=3))
    v2_pool = ctx.enter_context(tc.tile_pool(name="v2", bufs=3))
    out_pool = ctx.enter_context(tc.tile_pool(name="outp", bufs=4))
    fin_pool = ctx.enter_context(tc.tile_pool(name="fin", bufs=2))
    psac_pool = ctx.enter_context(tc.tile_pool(name="psac", bufs=2, space="PSUM"))
    psr_pool = ctx.enter_context(tc.tile_pool(name="psr", bufs=2, space="PSUM"))
    prs_pool = ctx.enter_context(tc.tile_pool(name="prs", bufs=3))

    sv_tiles = {}

    def stats_supertile(si):
        sv = sv_pool.tile([128, K * SV_W], f32)
        nc.sync.dma_start(sv[:], d_sv[:, ts(si, K * SV_W)])
        p1t = p1_pool.tile([128, K * BL], b16)
        nc.gpsimd.dma_start(p1t[:], d_p1[:, ts(si, K * BL)])
        scr = scr_pool.tile([128, K * SCW], b16)  # per j: s16 | s2 | vsq
        v2 = v2_pool.tile([128, K * VD], f32)
        sv3 = sv[:].rearrange("p (j w) -> p j w", w=SV_W)
        scr3 = scr[:].rearrange("p (j w) -> p j w", w=SCW)
        v23 = v2[:].rearrange("p (j w) -> p j w", w=VD)
        # casts + squares on ACT (merged across the supertile)
        nc.gpsimd.tensor_copy(scr3[:, :, 0:SDIM], sv3[:, :, 0:SDIM])
        nc.scalar.activation(scr3[:, :, SDIM:512], sv3[:, :, 0:SDIM], Act.Square)
        nc.scalar.activation(v23[:, :, :], sv3[:, :, SDIM:SV_W], Act.Square)
        # vsq = sum of 3 component squares (v is component-major)
        nc.gpsimd.tensor_add(scr3[:, :, 512:SCW], v23[:, :, 0:64],
                             v23[:, :, 64:128])
        nc.gpsimd.tensor_add(scr3[:, :, 512:SCW], scr3[:, :, 512:SCW],
                             v23[:, :, 128:192])
        for j in range(K):
            i = si * K + j
            st = (i == 0)
            # close the accumulation group at every chunk boundary so the
            # finalize PSUM read is legal in CoreSim (stop is a HW no-op);
            # later chunks keep accumulating with start=False.
            chunk_last = min((si // CH + 1) * CH, ST) * K - 1
            sp = (i == chunk_last)
            nc.tensor.matmul(ps_ss2[:], p1t[:, ts(j, BL)],
                             scr3[:, j, 0:512], start=st, stop=sp,
                             skip_group_check=True)
            nc.tensor.matmul(ps_v[:], p1t[:, ts(j, BL)],
                             scr3[:, j, 512:SCW], start=st, stop=sp,
                             skip_group_check=True)
        sv_tiles[si] = sv

    def finalize():
        # per-segment scale/offset from the (possibly still accumulating)
        # stats PSUM; rows of not-yet-seen segments are garbage-but-unused.
        me = fin_pool.tile([BL, 512], f32, tag="me")        # E[s] | E[s^2]
        nc.scalar.activation(me[:], ps_ss2[:], Act.Copy, scale=ci)
        smean = me[:, 0:SDIM]
        vm = fin_pool.tile([BL, BL], f32, tag="vm")         # E[|v|^2]
        nc.scalar.activation(vm[:], ps_v[:], Act.Copy, scale=ci)
        sm2 = fin_pool.tile([BL, SDIM], f32, tag="sm2")
        nc.scalar.activation(sm2[:], smean, Act.Square)
        varc = fin_pool.tile([BL, SDIM], f32, tag="varc")
        nc.vector.tensor_sub(varc[:], me[:, SDIM:512], sm2[:])
        nc.vector.tensor_scalar_max(varc[:], varc[:], EPS)
        rsq = fin_pool.tile([BL, SDIM], f32, tag="rsq")
        nc.vector.reciprocal(rsq[:], varc[:])
        y = fin_pool.tile([BL, SDIM], f32, tag="y")
        nc.scalar.sqrt(y[:], rsq[:])        # seed for 1/sqrt(var)
        tn = fin_pool.tile([BL, SDIM], f32, tag="tn")
        for _ in range(2):                  # Newton: y *= 1.5 - 0.5*var*y^2
            nc.vector.tensor_mul(tn[:], varc[:], y[:])
            nc.vector.tensor_mul(tn[:], tn[:], y[:])
            nc.scalar.activation(tn[:], tn[:], Act.Copy, scale=-0.5, bias=1.5)
            nc.vector.tensor_mul(y[:], y[:], tn[:])
        AC = fin_pool.tile([BL, 2 * SDIM], b16, tag="AC")
        nc.vector.tensor_mul(AC[:, 0:SDIM], y[:], wbt[:, 0:SDIM])
        mA = fin_pool.tile([BL, SDIM], f32, tag="mA")
        nc.vector.tensor_mul(mA[:], smean, AC[:, 0:SDIM])
        nc.vector.tensor_sub(AC[:, SDIM:2 * SDIM], wbt[:, SDIM:2 * SDIM], mA[:])
        vmc = fin_pool.tile([BL, BL], f32, tag="vmc")
        nc.vector.tensor_scalar_max(vmc[:], vm[:], EPS)
        rvf = fin_pool.tile([BL, BL], f32, tag="rvf")
        nc.vector.reciprocal(rvf[:], vmc[:])
        rv = fin_pool.tile([BL, BL], b16, tag="rv")
        nc.scalar.activation(rv[:], rvf[:], Act.Copy)
        return AC, rv

    def apply_supertile(si, AC, rv):
        sv = sv_tiles.pop(si)
        p3t = p3_pool.tile([BL, K * 128], b16)
        nc.gpsimd.dma_start(p3t[:], d_p3[:, ts(si, K * 128)])
        tout = out_pool.tile([128, K * SV_W], b16)
        sv3 = sv[:].rearrange("p (j w) -> p j w", w=SV_W)
        to3 = tout[:].rearrange("p (j w) -> p j w", w=SV_W)
        prs = prs_pool.tile([128, K * BL], f32)
        for h in range(K // 2):
            psac = psac_pool.tile([128, 1024], f32, tag="psac")
            psr = psr_pool.tile([128, 128], f32, tag="psr")
            for kk in range(2):
                j = 2 * h + kk
                nc.tensor.matmul(psac[:, ts(kk, 512)], p3t[:, ts(j, 128)],
                                 AC[:], start=True, stop=True)
                nc.tensor.matmul(psr[:, ts(kk, BL)], p3t[:, ts(j, 128)],
                                 rv[:], start=(kk == 0), stop=(kk == 1))
            nc.scalar.activation(prs[:, ts(h, 2 * BL)], psr[:], Act.Copy)
            pa3 = psac[:].rearrange("p (k w) -> p k w", w=512)
            o3 = to3[:, 2 * h:2 * h + 2, :]
            i3 = sv3[:, 2 * h:2 * h + 2, :]
            nc.vector.tensor_mul(o3[:, :, 0:SDIM], i3[:, :, 0:SDIM],
                                 pa3[:, :, 0:SDIM])
            nc.vector.tensor_add(o3[:, :, 0:SDIM], o3[:, :, 0:SDIM],
                                 pa3[:, :, SDIM:512])
        prb = prs[:].rearrange("p (k u w) -> p k u w", u=1,
                               w=BL).broadcast_to((128, K, 3, BL))
        nc.gpsimd.tensor_mul(to3[:, :, SDIM:SV_W], sv3[:, :, SDIM:SV_W], prb)
        nc.scalar.dma_start(d_out[:, ts(si, K * SV_W)], tout[:])

    def chunk_range(c):
        return range(c * CH, min((c + 1) * CH, ST))

    for si in chunk_range(0):
        stats_supertile(si)
    for c in range(NCHUNK):
        if c + 1 < NCHUNK:
            for si in chunk_range(c + 1):
                stats_supertile(si)
        AC, rv = finalize()
        for si in chunk_range(c):
            apply_supertile(si, AC, rv)


def _build():
    import concourse.bacc as bacc
    import concourse.tile as tile
    import concourse.mybir as mybir
    from contextlib import ExitStack

    nc = bacc.Bacc("TRN2", target_bir_lowering=False, debug=False,
                   num_devices=NCORES)
    d_sv = nc.dram_tensor("sv", [128, T * SV_W], mybir.dt.float32,
                          kind="ExternalInput").ap()
    d_p1 = nc.dram_tensor("p1", [128, T * BL], mybir.dt.bfloat16,
                          kind="ExternalInput").ap()
    d_p3 = nc.dram_tensor("p3", [BL, T * 128], mybir.dt.bfloat16,
                          kind="ExternalInput").ap()
    d_ci = nc.dram_tensor("ci", [BL, 1], mybir.dt.float32,
                          kind="ExternalInput").ap()
    d_wb = nc.dram_tensor("wb", [BL, 2 * SDIM], mybir.dt.float32,
                          kind="ExternalInput").ap()
    d_out = nc.dram_tensor("out", [128, T * SV_W], mybir.dt.bfloat16,
                           kind="ExternalOutput").ap()
    with tile.TileContext(nc) as tc:
        with ExitStack() as ctx:
            _emit(ctx, tc, nc, mybir, d_sv, d_p1, d_p3, d_ci, d_wb, d_out)
    nc.compile()
    return nc


def _get_compiled():
    global _compiled
    if _compiled is None:
        _compiled = _build()
    return _compiled


def _part_major(a, width):
    # [NT, width] node-major -> [128, T*width] partition-major supertile layout
    return np.ascontiguousarray(
        a.reshape(T, 128, width).transpose(1, 0, 2)).reshape(128, T * width)


LAST_RESULTS = None  # BassKernelResults of the most recent run (for profiling)


def prepare(s, v, batch, weight, bias):
    """Host-side sharding/staging. Returns (in_maps, metas)."""
    s = np.ascontiguousarray(np.asarray(s, dtype=np.float32))
    v = np.ascontiguousarray(np.asarray(v, dtype=np.float32))
    batch = np.asarray(batch).astype(np.int64)
    weight = np.asarray(weight, dtype=np.float32)
    bias = np.asarray(bias, dtype=np.float32)

    starts = np.searchsorted(batch, np.arange(0, B + 1, BL))
    cnt = np.bincount(batch, minlength=B).astype(np.float32)
    cnt_inv = (1.0 / np.maximum(cnt, 1.0)).astype(np.float32)
    wb = np.concatenate([np.tile(weight.reshape(1, SDIM), (BL, 1)),
                         np.tile(bias.reshape(1, SDIM), (BL, 1))],
                        axis=1).astype(np.float32)

    in_maps = []
    metas = []
    for c in range(NCORES):
        lo, hi = int(starts[c]), int(starts[c + 1])
        n = hi - lo
        assert n <= NT, f"core {c} shard {n} exceeds padded capacity {NT}"
        sf = np.zeros((NT, SDIM), dtype=np.float32)
        sf[:n] = s[lo:hi]
        vp = np.zeros((NT, VD), dtype=np.float32)
        vp[:n] = v[lo:hi].transpose(0, 2, 1).reshape(n, VD)  # component-major
        sv = _part_major(np.concatenate([sf, vp], axis=1), SV_W)
        segl = (batch[lo:hi] - c * BL).astype(np.int64)
        p1 = np.zeros((NT, BL), dtype=bf16)
        p1[np.arange(n), segl] = 1
        p1f = _part_major(p1, BL)
        p3f = np.ascontiguousarray(
            p1.reshape(T, 128, BL).transpose(2, 0, 1)).reshape(BL, T * 128)
        ci = cnt_inv[c * BL:(c + 1) * BL].reshape(BL, 1)
        in_maps.append({"sv": sv, "p1": p1f, "p3": p3f, "ci": ci, "wb": wb})
        metas.append((lo, n))
    return in_maps, metas


def gather(outs, metas, N):
    """Reassemble full outputs from per-core 'out' arrays."""
    sout = np.empty((N, SDIM), dtype=np.float32)
    vout = np.empty((N, VD // 3, 3), dtype=np.float32)
    for c, (lo, n) in enumerate(metas):
        o = np.asarray(outs[c]).astype(np.float32)
        o = o.reshape(128, T, SV_W).transpose(1, 0, 2).reshape(NT, SV_W)
        sout[lo:lo + n] = o[:n, 0:SDIM]
        vout[lo:lo + n] = o[:n, SDIM:SV_W].reshape(n, 3, VD // 3).transpose(0, 2, 1)
    return sout, vout


def kernel(s, v, batch, weight, bias):
    N = np.asarray(s).shape[0]
    in_maps, metas = prepare(s, v, batch, weight, bias)
    nc = _get_compiled()
    from concourse.bass_utils import run_bass_kernel_spmd
    res = run_bass_kernel_spmd(nc, in_maps, core_ids=list(range(NCORES)))
    global LAST_RESULTS
    LAST_RESULTS = res
    return gather([res.results[c]["out"] for c in range(NCORES)], metas, N)


# revision 13
# speedup vs baseline: 1.4359x; 1.4359x over previous
"""Segment-reduce BatchNorm (scalar + vector branches) on 8 TRN2 NeuronCores.

Strategy (per sharding hint): split the 512 sorted segments into 8 blocks of
64 contiguous segments; each core gets the nodes of its 64 segments, so all
segment statistics are device-local (no collectives).

Single-pass design: s and v stream through SBUF exactly once (fp32).  Node
tiles are processed in chunks; per-segment sums accumulate in PSUM via
one-hot bf16 matmuls while each chunk stays resident in SBUF.  Because batch
is sorted and a segment is far smaller than a chunk, the statistics of every
segment appearing in chunk c are final once chunk c+1's sums are in, so the
pipeline is: stats(c+1) -> finalize(c) -> apply(c), with apply reading the
retained SBUF tiles (no second DRAM read of s/v).

ACT casts s/v to bf16 and squares them; PE does all segment reductions and
per-node expansion of the per-segment scale/offset tables (bf16 one-hot
matmuls); DVE applies the normalization in fp32.

All host work is index/layout-only (sharding, padding, one-hot build, layout
transforms); every reduction and normalization happens on device.
"""

import numpy as np
import ml_dtypes

bf16 = ml_dtypes.bfloat16

B = 512           # total segments
NCORES = 8
BL = B // NCORES  # 64 segments per core
NT = 26624        # padded nodes per core (208 tiles of 128)
T = NT // 128     # 208 node tiles
K = 4             # node tiles per DMA supertile
ST = T // K       # 52 supertiles
CH = 8            # supertiles per chunk (4096 nodes >> max segment size)
NCHUNK = (ST + CH - 1) // CH
SDIM = 256
VD = 192          # 64 channels x 3 components, staged component-major
EPS = 1e-6
SV_W = SDIM + VD  # 448 cols per node: s | v
SCW = 576         # scratch cols per node: s16 | s2 | vsq

_compiled = None


def _emit(ctx, tc, nc, mybir, d_sv, d_p1, d_p3, d_ci, d_wb, d_out):
    import concourse.bass as bass

    f32 = mybir.dt.float32
    b16 = mybir.dt.bfloat16
    ts = bass.ts
    Act = mybir.ActivationFunctionType

    const_pool = ctx.enter_context(tc.tile_pool(name="const", bufs=1))
    cnti = const_pool.tile([BL, 1], f32)
    nc.sync.dma_start(cnti[:], d_ci[:])
    wbt = const_pool.tile([BL, 2 * SDIM], f32)
    nc.sync.dma_start(wbt[:], d_wb[:])
    ci = cnti[:, 0:1]

    # PSUM banks: stats 2 + psac 2x2 + psr 2x1 = 8
    pstat = ctx.enter_context(tc.tile_pool(name="pstat", bufs=1, space="PSUM"))
    ps_ss2 = pstat.tile([BL, 512], f32, tag="ps_ss2")   # sum s | sum s^2
    ps_v = pstat.tile([BL, BL], f32, tag="ps_v")        # sum |v|^2 per channel

    sv_pool = ctx.enter_context(tc.tile_pool(name="sv", bufs=2 * CH + 4))
    p1_pool = ctx.enter_context(tc.tile_pool(name="p1", bufs=3))
    p3_pool = ctx.enter_context(tc.tile_pool(name="p3", bufs=3))
    scr_pool = ctx.enter_context(tc.tile_pool(name="scr", bufs=3))
    v2_pool = ctx.enter_context(tc.tile_pool(name="v2", bufs=3))
    out_pool = ctx.enter_context(tc.tile_pool(name="outp", bufs=4))
    fin_pool = ctx.enter_context(tc.tile_pool(name="fin", bufs=2))
    psac_pool = ctx.enter_context(tc.tile_pool(name="psac", bufs=2, space="PSUM"))
    psr_pool = ctx.enter_context(tc.tile_pool(name="psr", bufs=2, space="PSUM"))

    sv_tiles = {}

    def stats_supertile(si):
        sv = sv_pool.tile([128, K * SV_W], f32)
        nc.sync.dma_start(sv[:], d_sv[:, ts(si, K * SV_W)])
        p1t = p1_pool.tile([128, K * BL], b16)
        nc.gpsimd.dma_start(p1t[:], d_p1[:, ts(si, K * BL)])
        scr = scr_pool.tile([128, K * SCW], b16)  # per j: s16 | s2 | vsq
        v2 = v2_pool.tile([128, K * VD], f32)
        sv3 = sv[:].rearrange("p (j w) -> p j w", w=SV_W)
        scr3 = scr[:].rearrange("p (j w) -> p j w", w=SCW)
        v23 = v2[:].rearrange("p (j w) -> p j w", w=VD)
        # casts + squares on ACT (merged across the supertile)
        nc.scalar.activation(scr3[:, :, 0:SDIM], sv3[:, :, 0:SDIM], Act.Copy)
        nc.scalar.activation(scr3[:, :, SDIM:512], sv3[:, :, 0:SDIM], Act.Square)
        nc.scalar.activation(v23[:, :, :], sv3[:, :, SDIM:SV_W], Act.Square)
        # vsq = sum of 3 component squares (v is component-major)
        nc.vector.tensor_add(scr3[:, :, 512:SCW], v23[:, :, 0:64],
                             v23[:, :, 64:128])
        nc.vector.tensor_add(scr3[:, :, 512:SCW], scr3[:, :, 512:SCW],
                             v23[:, :, 128:192])
        for j in range(K):
            i = si * K + j
            st = (i == 0)
            # close the accumulation group right before each finalize read so
            # it is legal in CoreSim (stop is a HW no-op); finalize for chunk
            # c runs after the FIRST supertile of chunk c+1 (one supertile >
            # max segment size), so groups close there and at the very end.
            sp = (i == T - 1) or (si % CH == 0 and si > 0 and i % K == K - 1)
            nc.tensor.matmul(ps_ss2[:], p1t[:, ts(j, BL)],
                             scr3[:, j, 0:512], start=st, stop=sp,
                             skip_group_check=True)
            nc.tensor.matmul(ps_v[:], p1t[:, ts(j, BL)],
                             scr3[:, j, 512:SCW], start=st, stop=sp,
                             skip_group_check=True)
        sv_tiles[si] = sv

    def finalize():
        # per-segment scale/offset from the (possibly still accumulating)
        # stats PSUM; rows of not-yet-seen segments are garbage-but-unused.
        me = fin_pool.tile([BL, 512], f32, tag="me")        # E[s] | E[s^2]
        nc.scalar.activation(me[:], ps_ss2[:], Act.Copy, scale=ci)
        smean = me[:, 0:SDIM]
        vm = fin_pool.tile([BL, BL], f32, tag="vm")         # E[|v|^2]
        nc.scalar.activation(vm[:], ps_v[:], Act.Copy, scale=ci)
        sm2 = fin_pool.tile([BL, SDIM], f32, tag="sm2")
        nc.scalar.activation(sm2[:], smean, Act.Square)
        varc = fin_pool.tile([BL, SDIM], f32, tag="varc")
        nc.vector.tensor_sub(varc[:], me[:, SDIM:512], sm2[:])
        nc.vector.tensor_scalar_max(varc[:], varc[:], EPS)
        rsq = fin_pool.tile([BL, SDIM], f32, tag="rsq")
        nc.vector.reciprocal(rsq[:], varc[:])
        y = fin_pool.tile([BL, SDIM], f32, tag="y")
        nc.scalar.sqrt(y[:], rsq[:])        # seed for 1/sqrt(var)
        tn = fin_pool.tile([BL, SDIM], f32, tag="tn")
        for _ in range(2):                  # Newton: y *= 1.5 - 0.5*var*y^2
            nc.vector.tensor_mul(tn[:], varc[:], y[:])
            nc.vector.tensor_mul(tn[:], tn[:], y[:])
            nc.scalar.activation(tn[:], tn[:], Act.Copy, scale=-0.5, bias=1.5)
            nc.vector.tensor_mul(y[:], y[:], tn[:])
        AC = fin_pool.tile([BL, 2 * SDIM], b16, tag="AC")
        nc.vector.tensor_mul(AC[:, 0:SDIM], y[:], wbt[:, 0:SDIM])
        mA = fin_pool.tile([BL, SDIM], f32, tag="mA")
        nc.vector.tensor_mul(mA[:], smean, AC[:, 0:SDIM])
        nc.vector.tensor_sub(AC[:, SDIM:2 * SDIM], wbt[:, SDIM:2 * SDIM], mA[:])
        vmc = fin_pool.tile([BL, BL], f32, tag="vmc")
        nc.vector.tensor_scalar_max(vmc[:], vm[:], EPS)
        rvf = fin_pool.tile([BL, BL], f32, tag="rvf")
        nc.vector.reciprocal(rvf[:], vmc[:])
        rv = fin_pool.tile([BL, BL], b16, tag="rv")
        nc.scalar.activation(rv[:], rvf[:], Act.Copy)
        return AC, rv

    def apply_supertile(si, AC, rv):
        sv = sv_tiles.pop(si)
        p3t = p3_pool.tile([BL, K * 128], b16)
        nc.gpsimd.dma_start(p3t[:], d_p3[:, ts(si, K * 128)])
        tout = out_pool.tile([128, K * SV_W], b16)
        sv3 = sv[:].rearrange("p (j w) -> p j w", w=SV_W)
        to3 = tout[:].rearrange("p (j w) -> p j w", w=SV_W)
        for h in range(K // 2):
            psac = psac_pool.tile([128, 1024], f32, tag="psac")
            psr = psr_pool.tile([128, 128], f32, tag="psr")
            for kk in range(2):
                j = 2 * h + kk
                nc.tensor.matmul(psac[:, ts(kk, 512)], p3t[:, ts(j, 128)],
                                 AC[:], start=True, stop=True)
                nc.tensor.matmul(psr[:, ts(kk, BL)], p3t[:, ts(j, 128)],
                                 rv[:], start=(kk == 0), stop=(kk == 1))
            pa3 = psac[:].rearrange("p (k w) -> p k w", w=512)
            pr3 = psr[:].rearrange("p (k w) -> p k w", w=BL)
            o3 = to3[:, 2 * h:2 * h + 2, :]
            i3 = sv3[:, 2 * h:2 * h + 2, :]
            nc.vector.tensor_mul(o3[:, :, 0:SDIM], i3[:, :, 0:SDIM],
                                 pa3[:, :, 0:SDIM])
            nc.vector.tensor_add(o3[:, :, 0:SDIM], o3[:, :, 0:SDIM],
                                 pa3[:, :, SDIM:512])
            prb = psr[:].rearrange("p (k u w) -> p k u w", u=1,
                                   w=BL).broadcast_to((128, 2, 3, BL))
            nc.vector.tensor_mul(o3[:, :, SDIM:SV_W], i3[:, :, SDIM:SV_W], prb)
        nc.scalar.dma_start(d_out[:, ts(si, K * SV_W)], tout[:])

    def chunk_range(c):
        return range(c * CH, min((c + 1) * CH, ST))

    for si in chunk_range(0):
        stats_supertile(si)
    for c in range(NCHUNK):
        nxt = list(chunk_range(c + 1))
        if nxt:
            stats_supertile(nxt[0])   # closes the PSUM group finalize reads
        AC, rv = finalize()
        cur = list(chunk_range(c))
        rest = nxt[1:]
        for idx in range(max(len(cur), len(rest))):
            if idx < len(rest):
                stats_supertile(rest[idx])
            if idx < len(cur):
                apply_supertile(cur[idx], AC, rv)


def _build():
    import concourse.bacc as bacc
    import concourse.tile as tile
    import concourse.mybir as mybir
    from contextlib import ExitStack

    nc = bacc.Bacc("TRN2", target_bir_lowering=False, debug=False,
                   num_devices=NCORES)
    d_sv = nc.dram_tensor("sv", [128, T * SV_W], mybir.dt.float32,
                          kind="ExternalInput").ap()
    d_p1 = nc.dram_tensor("p1", [128, T * BL], mybir.dt.bfloat16,
                          kind="ExternalInput").ap()
    d_p3 = nc.dram_tensor("p3", [BL, T * 128], mybir.dt.bfloat16,
                          kind="ExternalInput").ap()
    d_ci = nc.dram_tensor("ci", [BL, 1], mybir.dt.float32,
                          kind="ExternalInput").ap()
    d_wb = nc.dram_tensor("wb", [BL, 2 * SDIM], mybir.dt.float32,
                          kind="ExternalInput").ap()
    d_out = nc.dram_tensor("out", [128, T * SV_W], mybir.dt.bfloat16,
                           kind="ExternalOutput").ap()
    with tile.TileContext(nc) as tc:
        with ExitStack() as ctx:
            _emit(ctx, tc, nc, mybir, d_sv, d_p1, d_p3, d_ci, d_wb, d_out)
    nc.compile()
    return nc


def _get_compiled():
    global _compiled
    if _compiled is None:
        _compiled = _build()
    return _compiled


def _part_major(a, width):
    # [NT, width] node-major -> [128, T*width] partition-major supertile layout
    return np.ascontiguousarray(
        a.reshape(T, 128, width).transpose(1, 0, 2)).reshape(128, T * width)


LAST_RESULTS = None  # BassKernelResults of the most recent run (for profiling)


def prepare(s, v, batch, weight, bias):
    """Host-side sharding/staging. Returns (in_maps, metas)."""
    s = np.ascontiguousarray(np.asarray(s, dtype=np.float32))
    v = np.ascontiguousarray(np.asarray(v, dtype=np.float32))
    batch = np.asarray(batch).astype(np.int64)
    weight = np.asarray(weight, dtype=np.float32)
    bias = np.asarray(bias, dtype=np.float32)

    starts = np.searchsorted(batch, np.arange(0, B + 1, BL))
    cnt = np.bincount(batch, minlength=B).astype(np.float32)
    cnt_inv = (1.0 / np.maximum(cnt, 1.0)).astype(np.float32)
    wb = np.concatenate([np.tile(weight.reshape(1, SDIM), (BL, 1)),
                         np.tile(bias.reshape(1, SDIM), (BL, 1))],
                        axis=1).astype(np.float32)

    in_maps = []
    metas = []
    for c in range(NCORES):
        lo, hi = int(starts[c]), int(starts[c + 1])
        n = hi - lo
        assert n <= NT, f"core {c} shard {n} exceeds padded capacity {NT}"
        sf = np.zeros((NT, SDIM), dtype=np.float32)
        sf[:n] = s[lo:hi]
        vp = np.zeros((NT, VD), dtype=np.float32)
        vp[:n] = v[lo:hi].transpose(0, 2, 1).reshape(n, VD)  # component-major
        sv = _part_major(np.concatenate([sf, vp], axis=1), SV_W)
        segl = (batch[lo:hi] - c * BL).astype(np.int64)
        p1 = np.zeros((NT, BL), dtype=bf16)
        p1[np.arange(n), segl] = 1
        p1f = _part_major(p1, BL)
        p3f = np.ascontiguousarray(
            p1.reshape(T, 128, BL).transpose(2, 0, 1)).reshape(BL, T * 128)
        ci = cnt_inv[c * BL:(c + 1) * BL].reshape(BL, 1)
        in_maps.append({"sv": sv, "p1": p1f, "p3": p3f, "ci": ci, "wb": wb})
        metas.append((lo, n))
    return in_maps, metas


def gather(outs, metas, N):
    """Reassemble full outputs from per-core 'out' arrays."""
    sout = np.empty((N, SDIM), dtype=np.float32)
    vout = np.empty((N, VD // 3, 3), dtype=np.float32)
    for c, (lo, n) in enumerate(metas):
        o = np.asarray(outs[c]).astype(np.float32)
        o = o.reshape(128, T, SV_W).transpose(1, 0, 2).reshape(NT, SV_W)
        sout[lo:lo + n] = o[:n, 0:SDIM]
        vout[lo:lo + n] = o[:n, SDIM:SV_W].reshape(n, 3, VD // 3).transpose(0, 2, 1)
    return sout, vout


def kernel(s, v, batch, weight, bias):
    N = np.asarray(s).shape[0]
    in_maps, metas = prepare(s, v, batch, weight, bias)
    nc = _get_compiled()
    from concourse.bass_utils import run_bass_kernel_spmd
    res = run_bass_kernel_spmd(nc, in_maps, core_ids=list(range(NCORES)))
    global LAST_RESULTS
    LAST_RESULTS = res
    return gather([res.results[c]["out"] for c in range(NCORES)], metas, N)
